# revision 14
# baseline (speedup 1.0000x reference)
"""Trainium2 Bass kernel for a relu-RNN: h_t = relu(x_t @ W_xh^T + h_{t-1} @ W_hh^T + b_h).

Full shapes: x [64, 2048, 256], W_xh/W_hh [256, 256], b_h [256] -> out [64, 2048, 256] f32.

Strategy: data-parallel over batch (8 cores x 8 sequences), weights replicated.
Per core, hidden (256) lives on partitions as 2 chunks of 128; batch+time on the
free axis. Time is processed in blocks of TS steps: the input projection for a
block is computed by wide matmuls directly into PSUM, then TS sequential steps
accumulate W_hh @ h_{t-1} into the same PSUM columns and a single relu per step
moves PSUM -> SBUF h-history (which is both the next matmul operand and the
DMA-out source). The 8 local sequences are split into independent chains to
pipeline PE matmuls against the relu engines (ScalarE / VectorE alternating).
"""

import sys

for _p in ("/opt/trn_rl_repo",):
    if _p not in sys.path:
        sys.path.append(_p)

import numpy as np

import concourse.bass as bass
import concourse.tile as tile
from concourse import mybir
from concourse.bass_utils import run_bass_kernel_spmd
from concourse import bass_utils as _bass_utils

# Allow walrus to dedupe identical consecutive LDWEIGHTS: with the weight
# cycling order below, each chain's first matmul after the relu wait reuses
# the already-resident PE weights, so the (otherwise re-emitted) LDW would
# sit on the serial critical path.
_orig_run_command = _bass_utils.run_command


def _run_command_ldwopt(argv, **kwargs):
    argv = ["--enable-ldw-opt=true" if a == "--enable-ldw-opt=false" else a
            for a in argv]
    return _orig_run_command(argv, **kwargs)


def set_ldw_opt(enabled):
    _bass_utils.run_command = _run_command_ldwopt if enabled else _orig_run_command

# ---------------------------------------------------------------------------
# Workaround: this walrus build allows only ONE sync-wait command per
# instruction, but Tile attaches one wait per producer proc. Post-pass:
# move extra waits onto same-engine NOPs inserted just before the
# instruction (engine queues execute in program order, so blocking the
# queue on the NOP gives identical semantics).
_MAX_INST_WAITS = 1


def _split_multi_waits(nc):
    f = nc.m.functions[0]
    for bb in f.blocks:
        needs_fix = [
            i for i, inst in enumerate(bb.instructions)
            if inst.sync_info is not None
            and len(inst.sync_info.on_wait) > _MAX_INST_WAITS
        ]
        if not needs_fix:
            continue
        out = []
        for i, inst in enumerate(bb.instructions):
            si = inst.sync_info
            if si is not None and len(si.on_wait) > _MAX_INST_WAITS:
                waits = list(si.on_wait)
                # Keep the most meaningful wait on the instruction itself:
                # self-engine waits (pipeline-depth guards, satisfied long
                # before cross-engine data arrives) go to the NOP, so the
                # critical cross-engine wait does not pay the NOP dispatch
                # + inter-instruction bubble on the dependency edge.
                pfx = {"Pool": "POOL", "Activation": "ACT", "PE": "PE",
                       "DVE": "DVE", "SP": "SP"}.get(
                           str(inst.engine).split(".")[-1], "?")
                self_w = [w for w in waits
                          if str(w.ant_name).startswith(pfx + "_")]
                cross_w = [w for w in waits if w not in self_w]
                if cross_w:
                    keep = cross_w[-_MAX_INST_WAITS:]
                    extra = self_w + cross_w[:-_MAX_INST_WAITS]
                else:
                    keep = waits[-_MAX_INST_WAITS:]
                    extra = waits[:-_MAX_INST_WAITS]
                del si.on_wait[:]
                si.on_wait.extend(keep)
                for j in range(0, len(extra), _MAX_INST_WAITS):
                    nop = nc.engines[inst.engine].nop(hint="waitsplit").ins
                    # engine.nop() appended to the current bb; reclaim it
                    popped = nc.cur_bb.bb.instructions.pop()
                    assert popped is nop
                    nop.sync_info = mybir.SyncInfo(
                        on_wait=extra[j:j + _MAX_INST_WAITS], on_update=[])
                    out.append(nop)
            out.append(inst)
        bb.instructions = out

_WARM_LDW_NAMES = set()


def _dedupe_ldweights(nc):
    """Drop an InstLdweights whose weights exactly match the previous weight
    load on the PE stream (the following matmults carry ldweights=False and
    use the resident weights). Walrus re-emits every load otherwise, putting
    a ~100ns LDW on the serial critical path right after each relu wait."""
    f = nc.m.functions[0]
    removed = 0
    for bb in f.blocks:
        last_w = None
        out = []
        for inst in bb.instructions:
            tn = type(inst).__name__
            if tn == "InstLdweights":
                sig = (str(inst.ins[0]), str(inst.tile_position),
                       str(inst.perf_mode), str(inst.is_transpose))
                si = inst.sync_info
                clean = si is None or (not si.on_wait and not si.on_update)
                if inst.name in _WARM_LDW_NAMES:
                    # explicit warm-up load of the already-resident weights:
                    # keep it, and leave last_w so the following matmul's
                    # own (identical) load still dedupes.
                    if sig != last_w:
                        last_w = sig
                    out.append(inst)
                    continue
                if sig == last_w and clean:
                    removed += 1
                    continue
                last_w = sig
            elif tn == "InstMatmult":
                if inst.ldweights:
                    last_w = None  # self-loading matmul changes resident state
            out.append(inst)
        bb.instructions = out
    return removed
# ---------------------------------------------------------------------------

B, T, D, H = 64, 2048, 256, 256
N_CORES = 8
BPC = B // N_CORES  # sequences per core (8)

# Tunables
CHAINS = 2          # independent recurrent chains per core
TS = 64             # timesteps per block
DT = mybir.dt.float16
NP_DT = np.float16

BC = BPC // CHAINS  # sequences per chain
JW = TS * BC        # psum/h columns per chain per block
KC = D // 128       # contraction chunks (2)
MC = H // 128       # output-partition chunks (2)


def _build_program(t_total=T, chains=CHAINS, ts=TS, dt=DT,
                   relu_eng=None, dedupe_ldw=True, warm_ldw=True):
    tb_n = t_total // ts
    bc = BPC // chains
    jw = ts * bc
    if relu_eng is None:
        relu_eng = ["dve"] * chains
    _WARM_LDW_NAMES.clear()
    nc = bass.Bass("TRN2", target_bir_lowering=False, debug=False,
                   num_devices=N_CORES)

    x_d = nc.dram_tensor("x", [tb_n, KC, 128, chains, jw], dt,
                         kind="ExternalInput")
    whh_d = nc.dram_tensor("whh", [KC, MC, 128, 128], dt, kind="ExternalInput")
    wxh_d = nc.dram_tensor("wxh", [KC, MC, 128, 128], dt, kind="ExternalInput")
    b_d = nc.dram_tensor("b", [1, MC, 128], dt, kind="ExternalInput")
    out_d = nc.dram_tensor("out", [tb_n, chains, MC, 128, jw], dt,
                           kind="ExternalOutput")

    with tile.TileContext(nc) as tc:
        with (
            tc.tile_pool(name="consts", bufs=1) as consts,
            tc.tile_pool(name="xp", bufs=3) as xpool,
            tc.tile_pool(name="hp", bufs=4) as hpool,
            tc.tile_pool(name="pp", bufs=2, space=bass.MemorySpace.PSUM) as ppool,
        ):
            whh = consts.tile([128, KC, MC, 128], dt)
            nc.sync.dma_start(whh[:], whh_d.rearrange("k m p c -> p k m c"))
            wxh = consts.tile([128, KC, MC, 128], dt)
            nc.sync.dma_start(wxh[:], wxh_d.rearrange("k m p c -> p k m c"))
            bias = consts.tile([1, MC, 128], dt)
            nc.sync.dma_start(bias[:], b_d[:])
            ones = consts.tile([1, jw], dt)
            nc.vector.memset(ones[:], 1.0)

            wseq = [(m, k) for m in range(MC) for k in range(KC)]

            def alloc_block(tb):
                x_sb = xpool.tile([128, KC, chains, jw], dt, tag="x",
                                  name=f"x_{tb}")
                nc.sync.dma_start(x_sb[:],
                                  x_d[tb].rearrange("k p c j -> p k c j"))
                Hs = [hpool.tile([128, MC, jw], dt, tag=f"H{c}",
                                 name=f"H{c}_{tb}") for c in range(chains)]
                Ps = [ppool.tile([128, MC, jw], mybir.dt.float32, tag=f"P{c}",
                                 name=f"P{c}_{tb}") for c in range(chains)]
                return x_sb, Hs, Ps

            def xproj_ops(x_sb, Ps):
                # thunks for the input-projection matmuls of one block; the
                # caller drips them into the relu-wait windows of the
                # previous block so they never sit on the serial path.
                # start=True clears has_written for a whole PSUM bank ->
                # set it on the first matmul into each bank only.
                ops = []
                for c in range(chains):
                    seen = set()
                    for m in range(MC):
                        bank = (m * jw * 4) // 2048
                        first = bank not in seen
                        seen.add(bank)
                        ops.append((Ps[c][:, m, :], wxh[:, 0, m, :],
                                    x_sb[:, 0, c, :], first))
                        for k in range(1, KC):
                            ops.append((Ps[c][:, m, :], wxh[:, k, m, :],
                                        x_sb[:, k, c, :], False))
                        ops.append((Ps[c][:, m, :], bias[:, m, :],
                                    ones[:, :], False))
                return ops

            def emit_xproj(op):
                out_ap, lhs, rhs, first = op
                nc.tensor.matmul(out_ap, lhs, rhs, start=first, stop=False)

            cur = alloc_block(0)
            for op in xproj_ops(cur[0], cur[2]):
                emit_xproj(op)

            h_prev = [None] * chains
            for tb in range(tb_n):
                x_sb, Hs, Ps = cur
                pending = []
                if tb + 1 < tb_n:
                    nxt = alloc_block(tb + 1)
                    pending = list(xproj_ops(nxt[0], nxt[2]))
                else:
                    nxt = None

                for t in range(ts):
                    # drip one next-block input-projection matmul into this
                    # step's relu-wait window (PE is otherwise idle there)
                    if pending and t >= 4 and t % 8 == 4:
                        emit_xproj(pending.pop(0))
                    for c in range(chains):
                        # Alternate weight order every burst so the first
                        # matmul after the relu wait reuses the weights left
                        # resident by the previous burst (its LDW is then
                        # deduped away from the critical path).
                        fwd = (t * chains + c) % 2 == 0
                        order = wseq if fwd else wseq[::-1]
                        if not (tb == 0 and t == 0):
                            hp = (Hs[c][:, :, (t - 1) * bc:t * bc] if t > 0
                                  else h_prev[c][:, :, (ts - 1) * bc:ts * bc])
                            if warm_ldw:
                                m0, k0 = order[0]
                                wi = nc.tensor.ldweights(whh[:, k0, m0, :])
                                _WARM_LDW_NAMES.add(wi.ins.name)
                            for m, k in order:
                                last = (t == ts - 1 and (m, k) == order[-1])
                                nc.tensor.matmul(
                                    Ps[c][:, m, t * bc:(t + 1) * bc],
                                    whh[:, k, m, :], hp[:, k, :],
                                    start=False, stop=last)
                        dst = Hs[c][:, :, t * bc:(t + 1) * bc]
                        src = Ps[c][:, :, t * bc:(t + 1) * bc]
                        if relu_eng[c] == "act":
                            nc.scalar.activation(
                                dst, src, mybir.ActivationFunctionType.Relu)
                        else:
                            nc.vector.tensor_scalar_max(dst, src, 0.0)

                for op in pending:
                    emit_xproj(op)
                for c in range(chains):
                    nc.sync.dma_start(out_d[tb, c].rearrange("m p j -> p m j"),
                                      Hs[c][:])
                h_prev = Hs
                cur = nxt

    if dedupe_ldw:
        _dedupe_ldweights(nc)
    _split_multi_waits(nc)
    return nc


def _prep_inputs(x, W_xh, W_hh, b_h, t_total=T, chains=CHAINS, ts=TS,
                 np_dt=NP_DT):
    tb_n = t_total // ts
    bc = BPC // chains
    xr = x.reshape(N_CORES, chains, bc, tb_n, ts, KC, 128)
    # -> [core, tb, k, p, chain, ts, b]
    xp = np.ascontiguousarray(xr.transpose(0, 3, 5, 6, 1, 4, 2)).astype(np_dt)
    xp = xp.reshape(N_CORES, tb_n, KC, 128, chains, ts * bc)

    def wprep(w):
        # lhsT chunk [k][m][p][c] = W.T[128k+p, 128m+c] = W[128m+c, 128k+p]
        return np.ascontiguousarray(
            w.reshape(MC, 128, KC, 128).transpose(2, 0, 3, 1)).astype(np_dt)

    whh_p = wprep(W_hh)
    wxh_p = wprep(W_xh)
    b_p = b_h.reshape(1, MC, 128).astype(np_dt)
    in_maps = [
        {"x": xp[c], "whh": whh_p, "wxh": wxh_p, "b": b_p}
        for c in range(N_CORES)
    ]
    return in_maps


def _assemble_output(results, t_total=T, chains=CHAINS, ts=TS):
    tb_n = t_total // ts
    bc = BPC // chains
    outs = np.stack([results[c]["out"] for c in range(N_CORES)])
    # [core, tb, chain, m, p, ts, b] -> [core, chain, b, tb, ts, m, p]
    o = outs.reshape(N_CORES, tb_n, chains, MC, 128, ts, bc)
    o = o.transpose(0, 2, 6, 1, 5, 3, 4)
    return np.ascontiguousarray(o).reshape(B, t_total, H).astype(np.float32)


def run(x, W_xh, W_hh, b_h, *, t_total=T, chains=CHAINS, ts=TS, dt=DT,
        np_dt=NP_DT, trace=False, relu_eng=None, ldw_opt=False, dedupe_ldw=True,
        warm_ldw=True):
    set_ldw_opt(ldw_opt)
    nc = _build_program(t_total=t_total, chains=chains, ts=ts, dt=dt,
                        relu_eng=relu_eng, dedupe_ldw=dedupe_ldw,
                        warm_ldw=warm_ldw)
    in_maps = _prep_inputs(np.asarray(x), np.asarray(W_xh), np.asarray(W_hh),
                           np.asarray(b_h), t_total=t_total, chains=chains,
                           ts=ts, np_dt=np_dt)
    res = run_bass_kernel_spmd(nc, in_maps, list(range(N_CORES)), trace=trace)
    out = _assemble_output(res.results, t_total=t_total, chains=chains, ts=ts)
    return out, res


def kernel(x, W_xh, W_hh, b_h):
    out, _ = run(x, W_xh, W_hh, b_h)
    return out


# revision 15
# speedup vs baseline: 1.0792x; 1.0792x over previous
"""Trainium2 Bass kernel for a relu-RNN: h_t = relu(x_t @ W_xh^T + h_{t-1} @ W_hh^T + b_h).

Full shapes: x [64, 2048, 256], W_xh/W_hh [256, 256], b_h [256] -> out [64, 2048, 256] f32.

Strategy: data-parallel over batch (8 cores x 8 sequences), weights replicated.
Per core, hidden (256) lives on partitions as 2 chunks of 128; batch+time on the
free axis. Time is processed in blocks of TS steps: the input projection for a
block is computed by wide matmuls directly into PSUM, then TS sequential steps
accumulate W_hh @ h_{t-1} into the same PSUM columns and a single relu per step
moves PSUM -> SBUF h-history (which is both the next matmul operand and the
DMA-out source). The 8 local sequences are split into independent chains to
pipeline PE matmuls against the relu engines (ScalarE / VectorE alternating).
"""

import sys

for _p in ("/opt/trn_rl_repo",):
    if _p not in sys.path:
        sys.path.append(_p)

import numpy as np

import concourse.bass as bass
import concourse.tile as tile
from concourse import mybir
from concourse.bass_utils import run_bass_kernel_spmd
from concourse import bass_utils as _bass_utils

# Allow walrus to dedupe identical consecutive LDWEIGHTS: with the weight
# cycling order below, each chain's first matmul after the relu wait reuses
# the already-resident PE weights, so the (otherwise re-emitted) LDW would
# sit on the serial critical path.
_orig_run_command = _bass_utils.run_command


def _run_command_ldwopt(argv, **kwargs):
    argv = ["--enable-ldw-opt=true" if a == "--enable-ldw-opt=false" else a
            for a in argv]
    return _orig_run_command(argv, **kwargs)


def set_ldw_opt(enabled):
    _bass_utils.run_command = _run_command_ldwopt if enabled else _orig_run_command

# ---------------------------------------------------------------------------
# Workaround: this walrus build allows only ONE sync-wait command per
# instruction, but Tile attaches one wait per producer proc. Post-pass:
# move extra waits onto same-engine NOPs inserted just before the
# instruction (engine queues execute in program order, so blocking the
# queue on the NOP gives identical semantics).
_MAX_INST_WAITS = 1


def _split_multi_waits(nc):
    f = nc.m.functions[0]
    for bb in f.blocks:
        needs_fix = [
            i for i, inst in enumerate(bb.instructions)
            if inst.sync_info is not None
            and len(inst.sync_info.on_wait) > _MAX_INST_WAITS
        ]
        if not needs_fix:
            continue
        out = []
        for i, inst in enumerate(bb.instructions):
            si = inst.sync_info
            if si is not None and len(si.on_wait) > _MAX_INST_WAITS:
                waits = list(si.on_wait)
                # Keep the most meaningful wait on the instruction itself:
                # self-engine waits (pipeline-depth guards, satisfied long
                # before cross-engine data arrives) go to the NOP, so the
                # critical cross-engine wait does not pay the NOP dispatch
                # + inter-instruction bubble on the dependency edge.
                pfx = {"Pool": "POOL", "Activation": "ACT", "PE": "PE",
                       "DVE": "DVE", "SP": "SP"}.get(
                           str(inst.engine).split(".")[-1], "?")
                self_w = [w for w in waits
                          if str(w.ant_name).startswith(pfx + "_")]
                cross_w = [w for w in waits if w not in self_w]
                if cross_w:
                    keep = cross_w[-_MAX_INST_WAITS:]
                    extra = self_w + cross_w[:-_MAX_INST_WAITS]
                else:
                    keep = waits[-_MAX_INST_WAITS:]
                    extra = waits[:-_MAX_INST_WAITS]
                del si.on_wait[:]
                si.on_wait.extend(keep)
                for j in range(0, len(extra), _MAX_INST_WAITS):
                    nop = nc.engines[inst.engine].nop(hint="waitsplit").ins
                    # engine.nop() appended to the current bb; reclaim it
                    popped = nc.cur_bb.bb.instructions.pop()
                    assert popped is nop
                    nop.sync_info = mybir.SyncInfo(
                        on_wait=extra[j:j + _MAX_INST_WAITS], on_update=[])
                    out.append(nop)
            out.append(inst)
        bb.instructions = out

_WARM_LDW_NAMES = set()


def _dedupe_ldweights(nc):
    """Drop an InstLdweights whose weights exactly match the previous weight
    load on the PE stream (the following matmults carry ldweights=False and
    use the resident weights). Walrus re-emits every load otherwise, putting
    a ~100ns LDW on the serial critical path right after each relu wait."""
    f = nc.m.functions[0]
    removed = 0
    for bb in f.blocks:
        last_w = None
        out = []
        for inst in bb.instructions:
            tn = type(inst).__name__
            if tn == "InstLdweights":
                sig = (str(inst.ins[0]), str(inst.tile_position),
                       str(inst.perf_mode), str(inst.is_transpose))
                si = inst.sync_info
                clean = si is None or (not si.on_wait and not si.on_update)
                if inst.name in _WARM_LDW_NAMES:
                    # explicit warm-up load of the already-resident weights:
                    # keep it, and leave last_w so the following matmul's
                    # own (identical) load still dedupes.
                    if sig != last_w:
                        last_w = sig
                    out.append(inst)
                    continue
                if sig == last_w and clean:
                    removed += 1
                    continue
                last_w = sig
            elif tn == "InstMatmult":
                if inst.ldweights:
                    last_w = None  # self-loading matmul changes resident state
            out.append(inst)
        bb.instructions = out
    return removed
# ---------------------------------------------------------------------------

B, T, D, H = 64, 2048, 256, 256
N_CORES = 8
BPC = B // N_CORES  # sequences per core (8)

# Tunables
CHAINS = 2          # independent recurrent chains per core
TS = 64             # timesteps per block
DT = mybir.dt.float16
NP_DT = np.float16

BC = BPC // CHAINS  # sequences per chain
JW = TS * BC        # psum/h columns per chain per block
KC = D // 128       # contraction chunks (2)
MC = H // 128       # output-partition chunks (2)


def _build_program(t_total=T, chains=CHAINS, ts=TS, dt=DT,
                   relu_eng=None, dedupe_ldw=True, warm_ldw=False):
    tb_n = t_total // ts
    bc = BPC // chains
    jw = ts * bc
    if relu_eng is None:
        relu_eng = ["dve"] * chains
    _WARM_LDW_NAMES.clear()
    nc = bass.Bass("TRN2", target_bir_lowering=False, debug=False,
                   num_devices=N_CORES)

    x_d = nc.dram_tensor("x", [tb_n, KC, 128, chains, jw], dt,
                         kind="ExternalInput")
    whh_d = nc.dram_tensor("whh", [KC, MC, 128, 128], dt, kind="ExternalInput")
    wxh_d = nc.dram_tensor("wxh", [KC, MC, 128, 128], dt, kind="ExternalInput")
    b_d = nc.dram_tensor("b", [1, MC, 128], dt, kind="ExternalInput")
    out_d = nc.dram_tensor("out", [tb_n, chains, MC, 128, jw], dt,
                           kind="ExternalOutput")

    with tile.TileContext(nc) as tc:
        with (
            tc.tile_pool(name="consts", bufs=1) as consts,
            tc.tile_pool(name="xp", bufs=3) as xpool,
            tc.tile_pool(name="hp", bufs=4) as hpool,
            tc.tile_pool(name="pp", bufs=2, space=bass.MemorySpace.PSUM) as ppool,
        ):
            whh = consts.tile([128, KC, MC, 128], dt)
            nc.sync.dma_start(whh[:], whh_d.rearrange("k m p c -> p k m c"))
            wxh = consts.tile([128, KC, MC, 128], dt)
            nc.sync.dma_start(wxh[:], wxh_d.rearrange("k m p c -> p k m c"))
            bias = consts.tile([1, MC, 128], dt)
            nc.sync.dma_start(bias[:], b_d[:])
            ones = consts.tile([1, jw], dt)
            nc.vector.memset(ones[:], 1.0)

            wseq = [(m, k) for m in range(MC) for k in range(KC)]

            def alloc_block(tb):
                x_sb = xpool.tile([128, KC, chains, jw], dt, tag="x",
                                  name=f"x_{tb}")
                nc.sync.dma_start(x_sb[:],
                                  x_d[tb].rearrange("k p c j -> p k c j"))
                Hs = [hpool.tile([128, MC, jw], dt, tag=f"H{c}",
                                 name=f"H{c}_{tb}") for c in range(chains)]
                Ps = [ppool.tile([128, MC, jw], mybir.dt.float32, tag=f"P{c}",
                                 name=f"P{c}_{tb}") for c in range(chains)]
                return x_sb, Hs, Ps

            XPC = min(256, jw)  # xproj matmul chunk (fits a relu window)

            def xproj_ops(x_sb, Ps):
                # matmuls for one block's input projection, chunked to fit
                # the relu-wait windows of the previous block they get
                # dripped into. Chunks of one (m, k) pair are consecutive so
                # their repeated weight loads dedupe away.
                # start=True clears has_written for a whole PSUM bank ->
                # set it on the first matmul into each bank only.
                ops = []
                for c in range(chains):
                    seen = set()
                    for m in range(MC):
                        for k in range(KC):
                            for j0 in range(0, jw, XPC):
                                bank = (m * jw + j0) * 4 // 2048
                                first = bank not in seen
                                seen.add(bank)
                                ops.append((Ps[c][:, m, j0:j0 + XPC],
                                            wxh[:, k, m, :],
                                            x_sb[:, k, c, j0:j0 + XPC],
                                            first))
                        for j0 in range(0, jw, XPC):
                            ops.append((Ps[c][:, m, j0:j0 + XPC],
                                        bias[:, m, :],
                                        ones[:, j0:j0 + XPC], False))
                return ops

            def emit_xproj(op):
                out_ap, lhs, rhs, first = op
                nc.tensor.matmul(out_ap, lhs, rhs, start=first, stop=False)

            cur = alloc_block(0)
            for op in xproj_ops(cur[0], cur[2]):
                emit_xproj(op)

            h_prev = [None] * chains
            for tb in range(tb_n):
                x_sb, Hs, Ps = cur
                pending = []
                if tb + 1 < tb_n:
                    nxt = alloc_block(tb + 1)
                    pending = list(xproj_ops(nxt[0], nxt[2]))
                else:
                    nxt = None

                stride = max(2, ts // (len(pending) + 1)) if pending else ts
                for t in range(ts):
                    # drip one next-block input-projection matmul into this
                    # step's relu-wait window (PE is otherwise idle there)
                    if pending and t >= 2 and t % stride == 2 % stride:
                        emit_xproj(pending.pop(0))
                    for c in range(chains):
                        # Alternate weight order every burst so the first
                        # matmul after the relu wait reuses the weights left
                        # resident by the previous burst (its LDW is then
                        # deduped away from the critical path).
                        fwd = (t * chains + c) % 2 == 0
                        order = wseq if fwd else wseq[::-1]
                        if not (tb == 0 and t == 0):
                            hp = (Hs[c][:, :, (t - 1) * bc:t * bc] if t > 0
                                  else h_prev[c][:, :, (ts - 1) * bc:ts * bc])
                            if warm_ldw:
                                m0, k0 = order[0]
                                wi = nc.tensor.ldweights(whh[:, k0, m0, :])
                                _WARM_LDW_NAMES.add(wi.ins.name)
                            for m, k in order:
                                last = (t == ts - 1 and (m, k) == order[-1])
                                nc.tensor.matmul(
                                    Ps[c][:, m, t * bc:(t + 1) * bc],
                                    whh[:, k, m, :], hp[:, k, :],
                                    start=False, stop=last)
                        dst = Hs[c][:, :, t * bc:(t + 1) * bc]
                        src = Ps[c][:, :, t * bc:(t + 1) * bc]
                        if relu_eng[c] == "act":
                            nc.scalar.activation(
                                dst, src, mybir.ActivationFunctionType.Relu)
                        else:
                            nc.vector.tensor_scalar_max(dst, src, 0.0)

                for op in pending:
                    emit_xproj(op)
                for c in range(chains):
                    nc.sync.dma_start(out_d[tb, c].rearrange("m p j -> p m j"),
                                      Hs[c][:])
                h_prev = Hs
                cur = nxt

    if dedupe_ldw:
        _dedupe_ldweights(nc)
    _split_multi_waits(nc)
    return nc


def _prep_inputs(x, W_xh, W_hh, b_h, t_total=T, chains=CHAINS, ts=TS,
                 np_dt=NP_DT):
    tb_n = t_total // ts
    bc = BPC // chains
    xr = x.reshape(N_CORES, chains, bc, tb_n, ts, KC, 128)
    # -> [core, tb, k, p, chain, ts, b]
    xp = np.ascontiguousarray(xr.transpose(0, 3, 5, 6, 1, 4, 2)).astype(np_dt)
    xp = xp.reshape(N_CORES, tb_n, KC, 128, chains, ts * bc)

    def wprep(w):
        # lhsT chunk [k][m][p][c] = W.T[128k+p, 128m+c] = W[128m+c, 128k+p]
        return np.ascontiguousarray(
            w.reshape(MC, 128, KC, 128).transpose(2, 0, 3, 1)).astype(np_dt)

    whh_p = wprep(W_hh)
    wxh_p = wprep(W_xh)
    b_p = b_h.reshape(1, MC, 128).astype(np_dt)
    in_maps = [
        {"x": xp[c], "whh": whh_p, "wxh": wxh_p, "b": b_p}
        for c in range(N_CORES)
    ]
    return in_maps


def _assemble_output(results, t_total=T, chains=CHAINS, ts=TS):
    tb_n = t_total // ts
    bc = BPC // chains
    outs = np.stack([results[c]["out"] for c in range(N_CORES)])
    # [core, tb, chain, m, p, ts, b] -> [core, chain, b, tb, ts, m, p]
    o = outs.reshape(N_CORES, tb_n, chains, MC, 128, ts, bc)
    o = o.transpose(0, 2, 6, 1, 5, 3, 4)
    return np.ascontiguousarray(o).reshape(B, t_total, H).astype(np.float32)


def run(x, W_xh, W_hh, b_h, *, t_total=T, chains=CHAINS, ts=TS, dt=DT,
        np_dt=NP_DT, trace=False, relu_eng=None, ldw_opt=False, dedupe_ldw=True,
        warm_ldw=False):
    set_ldw_opt(ldw_opt)
    nc = _build_program(t_total=t_total, chains=chains, ts=ts, dt=dt,
                        relu_eng=relu_eng, dedupe_ldw=dedupe_ldw,
                        warm_ldw=warm_ldw)
    in_maps = _prep_inputs(np.asarray(x), np.asarray(W_xh), np.asarray(W_hh),
                           np.asarray(b_h), t_total=t_total, chains=chains,
                           ts=ts, np_dt=np_dt)
    res = run_bass_kernel_spmd(nc, in_maps, list(range(N_CORES)), trace=trace)
    out = _assemble_output(res.results, t_total=t_total, chains=chains, ts=ts)
    return out, res


def kernel(x, W_xh, W_hh, b_h):
    out, _ = run(x, W_xh, W_hh, b_h)
    return out


# revision 17
# speedup vs baseline: 1.0808x; 1.0015x over previous
"""Trainium2 Bass kernel for a relu-RNN: h_t = relu(x_t @ W_xh^T + h_{t-1} @ W_hh^T + b_h).

Full shapes: x [64, 2048, 256], W_xh/W_hh [256, 256], b_h [256] -> out [64, 2048, 256] f32.

Strategy: data-parallel over batch (8 cores x 8 sequences), weights replicated.
Per core, hidden (256) lives on partitions as 2 chunks of 128; batch+time on the
free axis. Time is processed in blocks of TS steps: the input projection for a
block is computed by wide matmuls directly into PSUM, then TS sequential steps
accumulate W_hh @ h_{t-1} into the same PSUM columns and a single relu per step
moves PSUM -> SBUF h-history (which is both the next matmul operand and the
DMA-out source). The 8 local sequences are split into independent chains to
pipeline PE matmuls against the relu engines (ScalarE / VectorE alternating).
"""

import sys

for _p in ("/opt/trn_rl_repo",):
    if _p not in sys.path:
        sys.path.append(_p)

import numpy as np

import concourse.bass as bass
import concourse.tile as tile
from concourse import mybir
from concourse.bass_utils import run_bass_kernel_spmd
from concourse import bass_utils as _bass_utils

# Allow walrus to dedupe identical consecutive LDWEIGHTS: with the weight
# cycling order below, each chain's first matmul after the relu wait reuses
# the already-resident PE weights, so the (otherwise re-emitted) LDW would
# sit on the serial critical path.
_orig_run_command = _bass_utils.run_command


def _run_command_ldwopt(argv, **kwargs):
    argv = ["--enable-ldw-opt=true" if a == "--enable-ldw-opt=false" else a
            for a in argv]
    return _orig_run_command(argv, **kwargs)


def set_ldw_opt(enabled):
    _bass_utils.run_command = _run_command_ldwopt if enabled else _orig_run_command

# ---------------------------------------------------------------------------
# Workaround: this walrus build allows only ONE sync-wait command per
# instruction, but Tile attaches one wait per producer proc. Post-pass:
# move extra waits onto same-engine NOPs inserted just before the
# instruction (engine queues execute in program order, so blocking the
# queue on the NOP gives identical semantics).
_MAX_INST_WAITS = 1


def _split_multi_waits(nc):
    f = nc.m.functions[0]
    for bb in f.blocks:
        needs_fix = [
            i for i, inst in enumerate(bb.instructions)
            if inst.sync_info is not None
            and len(inst.sync_info.on_wait) > _MAX_INST_WAITS
        ]
        if not needs_fix:
            continue
        out = []
        for i, inst in enumerate(bb.instructions):
            si = inst.sync_info
            if si is not None and len(si.on_wait) > _MAX_INST_WAITS:
                waits = list(si.on_wait)
                # Keep the most meaningful wait on the instruction itself:
                # self-engine waits (pipeline-depth guards, satisfied long
                # before cross-engine data arrives) go to the NOP, so the
                # critical cross-engine wait does not pay the NOP dispatch
                # + inter-instruction bubble on the dependency edge.
                pfx = {"Pool": "POOL", "Activation": "ACT", "PE": "PE",
                       "DVE": "DVE", "SP": "SP"}.get(
                           str(inst.engine).split(".")[-1], "?")
                self_w = [w for w in waits
                          if str(w.ant_name).startswith(pfx + "_")]
                cross_w = [w for w in waits if w not in self_w]
                if cross_w:
                    keep = cross_w[-_MAX_INST_WAITS:]
                    extra = self_w + cross_w[:-_MAX_INST_WAITS]
                else:
                    keep = waits[-_MAX_INST_WAITS:]
                    extra = waits[:-_MAX_INST_WAITS]
                del si.on_wait[:]
                si.on_wait.extend(keep)
                for j in range(0, len(extra), _MAX_INST_WAITS):
                    nop = nc.engines[inst.engine].nop(hint="waitsplit").ins
                    # engine.nop() appended to the current bb; reclaim it
                    popped = nc.cur_bb.bb.instructions.pop()
                    assert popped is nop
                    nop.sync_info = mybir.SyncInfo(
                        on_wait=extra[j:j + _MAX_INST_WAITS], on_update=[])
                    out.append(nop)
            out.append(inst)
        bb.instructions = out

_WARM_LDW_NAMES = set()


def _dedupe_ldweights(nc):
    """Drop an InstLdweights whose weights exactly match the previous weight
    load on the PE stream (the following matmults carry ldweights=False and
    use the resident weights). Walrus re-emits every load otherwise, putting
    a ~100ns LDW on the serial critical path right after each relu wait."""
    f = nc.m.functions[0]
    removed = 0
    for bb in f.blocks:
        last_w = None
        out = []
        for inst in bb.instructions:
            tn = type(inst).__name__
            if tn == "InstLdweights":
                sig = (str(inst.ins[0]), str(inst.tile_position),
                       str(inst.perf_mode), str(inst.is_transpose))
                si = inst.sync_info
                clean = si is None or (not si.on_wait and not si.on_update)
                if inst.name in _WARM_LDW_NAMES:
                    # explicit warm-up load of the already-resident weights:
                    # keep it, and leave last_w so the following matmul's
                    # own (identical) load still dedupes.
                    if sig != last_w:
                        last_w = sig
                    out.append(inst)
                    continue
                if sig == last_w and clean:
                    removed += 1
                    continue
                last_w = sig
            elif tn == "InstMatmult":
                if inst.ldweights:
                    last_w = None  # self-loading matmul changes resident state
            out.append(inst)
        bb.instructions = out
    return removed
# ---------------------------------------------------------------------------

B, T, D, H = 64, 2048, 256, 256
N_CORES = 8
BPC = B // N_CORES  # sequences per core (8)

# Tunables
CHAINS = 2          # independent recurrent chains per core
TS = 64             # timesteps per block
DT = mybir.dt.float16
NP_DT = np.float16

BC = BPC // CHAINS  # sequences per chain
JW = TS * BC        # psum/h columns per chain per block
KC = D // 128       # contraction chunks (2)
MC = H // 128       # output-partition chunks (2)


def _build_program(t_total=T, chains=CHAINS, ts=TS, dt=DT,
                   relu_eng=None, dedupe_ldw=True, warm_ldw=False):
    tb_n = t_total // ts
    bc = BPC // chains
    jw = ts * bc
    if relu_eng is None:
        relu_eng = ["dve"] * chains
    _WARM_LDW_NAMES.clear()
    nc = bass.Bass("TRN2", target_bir_lowering=False, debug=False,
                   num_devices=N_CORES)

    x_d = nc.dram_tensor("x", [tb_n, KC, 128, chains, jw], dt,
                         kind="ExternalInput")
    whh_d = nc.dram_tensor("whh", [KC, MC, 128, 128], dt, kind="ExternalInput")
    wxh_d = nc.dram_tensor("wxh", [KC, MC, 128, 128], dt, kind="ExternalInput")
    b_d = nc.dram_tensor("b", [1, MC, 128], dt, kind="ExternalInput")
    out_d = nc.dram_tensor("out", [tb_n, chains, MC, 128, jw], dt,
                           kind="ExternalOutput")

    with tile.TileContext(nc) as tc:
        with (
            tc.tile_pool(name="consts", bufs=1) as consts,
            tc.tile_pool(name="xp", bufs=3) as xpool,
            tc.tile_pool(name="hp", bufs=4) as hpool,
            tc.tile_pool(name="pp", bufs=2, space=bass.MemorySpace.PSUM) as ppool,
        ):
            whh = consts.tile([128, KC, MC, 128], dt)
            nc.sync.dma_start(whh[:], whh_d.rearrange("k m p c -> p k m c"))
            wxh = consts.tile([128, KC, MC, 128], dt)
            nc.sync.dma_start(wxh[:], wxh_d.rearrange("k m p c -> p k m c"))
            bias = consts.tile([1, MC, 128], dt)
            nc.sync.dma_start(bias[:], b_d[:])
            ones = consts.tile([1, jw], dt)
            nc.vector.memset(ones[:], 1.0)

            wseq = [(m, k) for m in range(MC) for k in range(KC)]

            def alloc_block(tb):
                x_sb = xpool.tile([128, KC, chains, jw], dt, tag="x",
                                  name=f"x_{tb}")
                nc.sync.dma_start(x_sb[:],
                                  x_d[tb].rearrange("k p c j -> p k c j"))
                Hs = [hpool.tile([128, MC, jw], dt, tag=f"H{c}",
                                 name=f"H{c}_{tb}") for c in range(chains)]
                Ps = [ppool.tile([128, MC, jw], mybir.dt.float32, tag=f"P{c}",
                                 name=f"P{c}_{tb}") for c in range(chains)]
                return x_sb, Hs, Ps

            XPC = min(256, jw)  # xproj matmul chunk (fits a relu window)

            def xproj_ops(x_sb, Ps):
                # matmuls for one block's input projection, chunked to fit
                # the relu-wait windows of the previous block they get
                # dripped into. Chunks of one (m, k) pair are consecutive so
                # their repeated weight loads dedupe away.
                # start=True clears has_written for a whole PSUM bank ->
                # set it on the first matmul into each bank only.
                ops = []
                for c in range(chains):
                    seen = set()
                    for m in range(MC):
                        for k in range(KC):
                            for j0 in range(0, jw, XPC):
                                bank = (m * jw + j0) * 4 // 2048
                                first = bank not in seen
                                seen.add(bank)
                                ops.append((Ps[c][:, m, j0:j0 + XPC],
                                            wxh[:, k, m, :],
                                            x_sb[:, k, c, j0:j0 + XPC],
                                            first))
                        for j0 in range(0, jw, XPC):
                            ops.append((Ps[c][:, m, j0:j0 + XPC],
                                        bias[:, m, :],
                                        ones[:, j0:j0 + XPC], False))
                return ops

            def emit_xproj(op):
                out_ap, lhs, rhs, first = op
                return nc.tensor.matmul(out_ap, lhs, rhs, start=first,
                                        stop=False)

            cur = alloc_block(0)
            for op in xproj_ops(cur[0], cur[2]):
                emit_xproj(op)

            h_prev = [None] * chains
            for tb in range(tb_n):
                x_sb, Hs, Ps = cur
                pending = []
                if tb + 1 < tb_n:
                    nxt = alloc_block(tb + 1)
                    pending = list(xproj_ops(nxt[0], nxt[2]))
                else:
                    nxt = None

                stride = max(2, ts // (len(pending) + 1)) if pending else ts
                for t in range(ts):
                    last_mm = None
                    for c in range(chains):
                        # Alternate weight order every burst so the first
                        # matmul after the relu wait reuses the weights left
                        # resident by the previous burst (its LDW is then
                        # deduped away from the critical path).
                        fwd = (t * chains + c) % 2 == 0
                        order = wseq if fwd else wseq[::-1]
                        if not (tb == 0 and t == 0):
                            hp = (Hs[c][:, :, (t - 1) * bc:t * bc] if t > 0
                                  else h_prev[c][:, :, (ts - 1) * bc:ts * bc])
                            if warm_ldw:
                                m0, k0 = order[0]
                                wi = nc.tensor.ldweights(whh[:, k0, m0, :])
                                _WARM_LDW_NAMES.add(wi.ins.name)
                            for m, k in order:
                                last = (t == ts - 1 and (m, k) == order[-1])
                                last_mm = nc.tensor.matmul(
                                    Ps[c][:, m, t * bc:(t + 1) * bc],
                                    whh[:, k, m, :], hp[:, k, :],
                                    start=False, stop=last)
                        dst = Hs[c][:, :, t * bc:(t + 1) * bc]
                        src = Ps[c][:, :, t * bc:(t + 1) * bc]
                        if relu_eng[c] == "act":
                            nc.scalar.activation(
                                dst, src, mybir.ActivationFunctionType.Relu)
                        else:
                            nc.vector.tensor_scalar_max(dst, src, 0.0)

                    # drip one next-block input-projection matmul into this
                    # step's relu-wait window, pinned right after this
                    # step's matmuls in the PE queue (the scheduler would
                    # otherwise clump them at the end of the block)
                    if (pending and t >= 2 and t % stride == 2 % stride
                            and last_mm is not None):
                        xp_h = emit_xproj(pending.pop(0))
                        tile.add_dep_helper(last_mm.ins, xp_h.ins, sync=False,
                                            reason="fill relu window")

                for op in pending:
                    emit_xproj(op)
                for c in range(chains):
                    nc.sync.dma_start(out_d[tb, c].rearrange("m p j -> p m j"),
                                      Hs[c][:])
                h_prev = Hs
                cur = nxt

    if dedupe_ldw:
        _dedupe_ldweights(nc)
    _split_multi_waits(nc)
    return nc


def _prep_inputs(x, W_xh, W_hh, b_h, t_total=T, chains=CHAINS, ts=TS,
                 np_dt=NP_DT):
    tb_n = t_total // ts
    bc = BPC // chains
    xr = x.reshape(N_CORES, chains, bc, tb_n, ts, KC, 128)
    # -> [core, tb, k, p, chain, ts, b]
    xp = np.ascontiguousarray(xr.transpose(0, 3, 5, 6, 1, 4, 2)).astype(np_dt)
    xp = xp.reshape(N_CORES, tb_n, KC, 128, chains, ts * bc)

    def wprep(w):
        # lhsT chunk [k][m][p][c] = W.T[128k+p, 128m+c] = W[128m+c, 128k+p]
        return np.ascontiguousarray(
            w.reshape(MC, 128, KC, 128).transpose(2, 0, 3, 1)).astype(np_dt)

    whh_p = wprep(W_hh)
    wxh_p = wprep(W_xh)
    b_p = b_h.reshape(1, MC, 128).astype(np_dt)
    in_maps = [
        {"x": xp[c], "whh": whh_p, "wxh": wxh_p, "b": b_p}
        for c in range(N_CORES)
    ]
    return in_maps


def _assemble_output(results, t_total=T, chains=CHAINS, ts=TS):
    tb_n = t_total // ts
    bc = BPC // chains
    outs = np.stack([results[c]["out"] for c in range(N_CORES)])
    # [core, tb, chain, m, p, ts, b] -> [core, chain, b, tb, ts, m, p]
    o = outs.reshape(N_CORES, tb_n, chains, MC, 128, ts, bc)
    o = o.transpose(0, 2, 6, 1, 5, 3, 4)
    return np.ascontiguousarray(o).reshape(B, t_total, H).astype(np.float32)


def run(x, W_xh, W_hh, b_h, *, t_total=T, chains=CHAINS, ts=TS, dt=DT,
        np_dt=NP_DT, trace=False, relu_eng=None, ldw_opt=False, dedupe_ldw=True,
        warm_ldw=False):
    set_ldw_opt(ldw_opt)
    nc = _build_program(t_total=t_total, chains=chains, ts=ts, dt=dt,
                        relu_eng=relu_eng, dedupe_ldw=dedupe_ldw,
                        warm_ldw=warm_ldw)
    in_maps = _prep_inputs(np.asarray(x), np.asarray(W_xh), np.asarray(W_hh),
                           np.asarray(b_h), t_total=t_total, chains=chains,
                           ts=ts, np_dt=np_dt)
    res = run_bass_kernel_spmd(nc, in_maps, list(range(N_CORES)), trace=trace)
    out = _assemble_output(res.results, t_total=t_total, chains=chains, ts=ts)
    return out, res


def kernel(x, W_xh, W_hh, b_h):
    # The device pool occasionally throws a transient
    # NRT_EXEC_UNIT_UNRECOVERABLE on execute; a fresh attempt succeeds.
    last_err = None
    for _ in range(3):
        try:
            out, _ = run(x, W_xh, W_hh, b_h)
            return out
        except Exception as e:  # noqa: BLE001
            last_err = e
            import time as _time
            _time.sleep(5)
    raise last_err


# revision 19
# speedup vs baseline: 1.0851x; 1.0040x over previous
"""Trainium2 Bass kernel for a relu-RNN: h_t = relu(x_t @ W_xh^T + h_{t-1} @ W_hh^T + b_h).

Full shapes: x [64, 2048, 256], W_xh/W_hh [256, 256], b_h [256] -> out [64, 2048, 256] f32.

Strategy: data-parallel over batch (8 cores x 8 sequences), weights replicated.
Per core, hidden (256) lives on partitions as 2 chunks of 128; batch+time on the
free axis. Time is processed in blocks of TS steps: the input projection for a
block is computed by wide matmuls directly into PSUM, then TS sequential steps
accumulate W_hh @ h_{t-1} into the same PSUM columns and a single relu per step
moves PSUM -> SBUF h-history (which is both the next matmul operand and the
DMA-out source). The 8 local sequences are split into independent chains to
pipeline PE matmuls against the relu engines (ScalarE / VectorE alternating).
"""

import sys

for _p in ("/opt/trn_rl_repo",):
    if _p not in sys.path:
        sys.path.append(_p)

import numpy as np

import concourse.bass as bass
import concourse.tile as tile
from concourse import mybir
from concourse.bass_utils import run_bass_kernel_spmd
from concourse import bass_utils as _bass_utils

# Allow walrus to dedupe identical consecutive LDWEIGHTS: with the weight
# cycling order below, each chain's first matmul after the relu wait reuses
# the already-resident PE weights, so the (otherwise re-emitted) LDW would
# sit on the serial critical path.
_orig_run_command = _bass_utils.run_command


def _run_command_ldwopt(argv, **kwargs):
    argv = ["--enable-ldw-opt=true" if a == "--enable-ldw-opt=false" else a
            for a in argv]
    return _orig_run_command(argv, **kwargs)


def set_ldw_opt(enabled):
    _bass_utils.run_command = _run_command_ldwopt if enabled else _orig_run_command

# ---------------------------------------------------------------------------
# Workaround: this walrus build allows only ONE sync-wait command per
# instruction, but Tile attaches one wait per producer proc. Post-pass:
# move extra waits onto same-engine NOPs inserted just before the
# instruction (engine queues execute in program order, so blocking the
# queue on the NOP gives identical semantics).
_MAX_INST_WAITS = 1


def _split_multi_waits(nc):
    f = nc.m.functions[0]
    for bb in f.blocks:
        needs_fix = [
            i for i, inst in enumerate(bb.instructions)
            if inst.sync_info is not None
            and len(inst.sync_info.on_wait) > _MAX_INST_WAITS
        ]
        if not needs_fix:
            continue
        out = []
        for i, inst in enumerate(bb.instructions):
            si = inst.sync_info
            if si is not None and len(si.on_wait) > _MAX_INST_WAITS:
                waits = list(si.on_wait)
                # Keep the most meaningful wait on the instruction itself:
                # self-engine waits (pipeline-depth guards, satisfied long
                # before cross-engine data arrives) go to the NOP, so the
                # critical cross-engine wait does not pay the NOP dispatch
                # + inter-instruction bubble on the dependency edge.
                pfx = {"Pool": "POOL", "Activation": "ACT", "PE": "PE",
                       "DVE": "DVE", "SP": "SP"}.get(
                           str(inst.engine).split(".")[-1], "?")
                self_w = [w for w in waits
                          if str(w.ant_name).startswith(pfx + "_")]
                cross_w = [w for w in waits if w not in self_w]
                if cross_w:
                    keep = cross_w[-_MAX_INST_WAITS:]
                    extra = self_w + cross_w[:-_MAX_INST_WAITS]
                else:
                    keep = waits[-_MAX_INST_WAITS:]
                    extra = waits[:-_MAX_INST_WAITS]
                del si.on_wait[:]
                si.on_wait.extend(keep)
                for j in range(0, len(extra), _MAX_INST_WAITS):
                    nop = nc.engines[inst.engine].nop(hint="waitsplit").ins
                    # engine.nop() appended to the current bb; reclaim it
                    popped = nc.cur_bb.bb.instructions.pop()
                    assert popped is nop
                    nop.sync_info = mybir.SyncInfo(
                        on_wait=extra[j:j + _MAX_INST_WAITS], on_update=[])
                    out.append(nop)
            out.append(inst)
        bb.instructions = out

_WARM_LDW_NAMES = set()


def _dedupe_ldweights(nc):
    """Drop an InstLdweights whose weights exactly match the previous weight
    load on the PE stream (the following matmults carry ldweights=False and
    use the resident weights). Walrus re-emits every load otherwise, putting
    a ~100ns LDW on the serial critical path right after each relu wait."""
    f = nc.m.functions[0]
    removed = 0
    for bb in f.blocks:
        last_w = None
        out = []
        for inst in bb.instructions:
            tn = type(inst).__name__
            if tn == "InstLdweights":
                sig = (str(inst.ins[0]), str(inst.tile_position),
                       str(inst.perf_mode), str(inst.is_transpose))
                si = inst.sync_info
                clean = si is None or (not si.on_wait and not si.on_update)
                if inst.name in _WARM_LDW_NAMES:
                    # explicit warm-up load of the already-resident weights:
                    # keep it, and leave last_w so the following matmul's
                    # own (identical) load still dedupes.
                    if sig != last_w:
                        last_w = sig
                    out.append(inst)
                    continue
                if sig == last_w and clean:
                    removed += 1
                    continue
                last_w = sig
            elif tn == "InstMatmult":
                if inst.ldweights:
                    last_w = None  # self-loading matmul changes resident state
            out.append(inst)
        bb.instructions = out
    return removed
# ---------------------------------------------------------------------------

B, T, D, H = 64, 2048, 256, 256
N_CORES = 8
BPC = B // N_CORES  # sequences per core (8)

# Tunables
CHAINS = 1          # independent recurrent chains per core
TS = 64             # timesteps per block
DT = mybir.dt.float16
NP_DT = np.float16

BC = BPC // CHAINS  # sequences per chain
JW = TS * BC        # psum/h columns per chain per block
KC = D // 128       # contraction chunks (2)
MC = H // 128       # output-partition chunks (2)


def _build_program(t_total=T, chains=CHAINS, ts=TS, dt=DT,
                   relu_eng=None, dedupe_ldw=True, warm_ldw=False):
    tb_n = t_total // ts
    bc = BPC // chains
    jw = ts * bc
    if relu_eng is None:
        relu_eng = ["dve"] * chains
    _WARM_LDW_NAMES.clear()
    nc = bass.Bass("TRN2", target_bir_lowering=False, debug=False,
                   num_devices=N_CORES)

    x_d = nc.dram_tensor("x", [tb_n, KC, 128, chains, jw], dt,
                         kind="ExternalInput")
    whh_d = nc.dram_tensor("whh", [KC, MC, 128, 128], dt, kind="ExternalInput")
    wxh_d = nc.dram_tensor("wxh", [KC, MC, 128, 128], dt, kind="ExternalInput")
    b_d = nc.dram_tensor("b", [1, MC, 128], dt, kind="ExternalInput")
    out_d = nc.dram_tensor("out", [tb_n, chains, MC, 128, jw], dt,
                           kind="ExternalOutput")

    with tile.TileContext(nc) as tc:
        with (
            tc.tile_pool(name="consts", bufs=1) as consts,
            tc.tile_pool(name="xp", bufs=3) as xpool,
            tc.tile_pool(name="hp", bufs=4) as hpool,
            tc.tile_pool(name="pp", bufs=2, space=bass.MemorySpace.PSUM) as ppool,
        ):
            whh = consts.tile([128, KC, MC, 128], dt)
            nc.sync.dma_start(whh[:], whh_d.rearrange("k m p c -> p k m c"))
            wxh = consts.tile([128, KC, MC, 128], dt)
            nc.sync.dma_start(wxh[:], wxh_d.rearrange("k m p c -> p k m c"))
            bias = consts.tile([1, MC, 128], dt)
            nc.sync.dma_start(bias[:], b_d[:])
            ones = consts.tile([1, jw], dt)
            nc.vector.memset(ones[:], 1.0)

            wseq = [(m, k) for m in range(MC) for k in range(KC)]

            def alloc_block(tb):
                x_sb = xpool.tile([128, KC, chains, jw], dt, tag="x",
                                  name=f"x_{tb}")
                nc.sync.dma_start(x_sb[:],
                                  x_d[tb].rearrange("k p c j -> p k c j"))
                Hs = [hpool.tile([128, MC, jw], dt, tag=f"H{c}",
                                 name=f"H{c}_{tb}") for c in range(chains)]
                Ps = [ppool.tile([128, MC, jw], mybir.dt.float32, tag=f"P{c}",
                                 name=f"P{c}_{tb}") for c in range(chains)]
                return x_sb, Hs, Ps

            XPC = min(256, jw)  # xproj matmul chunk (fits a relu window)

            def xproj_ops(x_sb, Ps):
                # matmuls for one block's input projection, chunked to fit
                # the relu-wait windows of the previous block they get
                # dripped into. Chunks of one (m, k) pair are consecutive so
                # their repeated weight loads dedupe away.
                # start=True clears has_written for a whole PSUM bank ->
                # set it on the first matmul into each bank only.
                ops = []
                for c in range(chains):
                    seen = set()
                    for m in range(MC):
                        for k in range(KC):
                            for j0 in range(0, jw, XPC):
                                bank = (m * jw + j0) * 4 // 2048
                                first = bank not in seen
                                seen.add(bank)
                                ops.append((Ps[c][:, m, j0:j0 + XPC],
                                            wxh[:, k, m, :],
                                            x_sb[:, k, c, j0:j0 + XPC],
                                            first))
                        for j0 in range(0, jw, XPC):
                            ops.append((Ps[c][:, m, j0:j0 + XPC],
                                        bias[:, m, :],
                                        ones[:, j0:j0 + XPC], False))
                return ops

            def emit_xproj(op):
                out_ap, lhs, rhs, first = op
                return nc.tensor.matmul(out_ap, lhs, rhs, start=first,
                                        stop=False)

            cur = alloc_block(0)
            for op in xproj_ops(cur[0], cur[2]):
                emit_xproj(op)

            h_prev = [None] * chains
            for tb in range(tb_n):
                x_sb, Hs, Ps = cur
                pending = []
                if tb + 1 < tb_n:
                    nxt = alloc_block(tb + 1)
                    pending = list(xproj_ops(nxt[0], nxt[2]))
                else:
                    nxt = None

                stride = max(2, ts // (len(pending) + 1)) if pending else ts
                for t in range(ts):
                    last_mm = None
                    for c in range(chains):
                        # Alternate weight order every burst so the first
                        # matmul after the relu wait reuses the weights left
                        # resident by the previous burst (its LDW is then
                        # deduped away from the critical path).
                        fwd = (t * chains + c) % 2 == 0
                        order = wseq if fwd else wseq[::-1]
                        if not (tb == 0 and t == 0):
                            hp = (Hs[c][:, :, (t - 1) * bc:t * bc] if t > 0
                                  else h_prev[c][:, :, (ts - 1) * bc:ts * bc])
                            if warm_ldw:
                                m0, k0 = order[0]
                                wi = nc.tensor.ldweights(whh[:, k0, m0, :])
                                _WARM_LDW_NAMES.add(wi.ins.name)
                            for m, k in order:
                                last = (t == ts - 1 and (m, k) == order[-1])
                                last_mm = nc.tensor.matmul(
                                    Ps[c][:, m, t * bc:(t + 1) * bc],
                                    whh[:, k, m, :], hp[:, k, :],
                                    start=False, stop=last)
                        dst = Hs[c][:, :, t * bc:(t + 1) * bc]
                        src = Ps[c][:, :, t * bc:(t + 1) * bc]
                        if relu_eng[c] == "act":
                            nc.scalar.activation(
                                dst, src, mybir.ActivationFunctionType.Relu)
                        else:
                            nc.vector.tensor_scalar_max(dst, src, 0.0)

                    # drip one next-block input-projection matmul into this
                    # step's relu-wait window, pinned right after this
                    # step's matmuls in the PE queue (the scheduler would
                    # otherwise clump them at the end of the block)
                    if (pending and t >= 2 and t % stride == 2 % stride
                            and last_mm is not None):
                        xp_h = emit_xproj(pending.pop(0))
                        tile.add_dep_helper(last_mm.ins, xp_h.ins, sync=False,
                                            reason="fill relu window")

                for op in pending:
                    emit_xproj(op)
                for c in range(chains):
                    nc.sync.dma_start(out_d[tb, c].rearrange("m p j -> p m j"),
                                      Hs[c][:])
                h_prev = Hs
                cur = nxt

    if dedupe_ldw:
        _dedupe_ldweights(nc)
    _split_multi_waits(nc)
    return nc


def _prep_inputs(x, W_xh, W_hh, b_h, t_total=T, chains=CHAINS, ts=TS,
                 np_dt=NP_DT):
    tb_n = t_total // ts
    bc = BPC // chains
    xr = x.reshape(N_CORES, chains, bc, tb_n, ts, KC, 128)
    # -> [core, tb, k, p, chain, ts, b]
    xp = np.ascontiguousarray(xr.transpose(0, 3, 5, 6, 1, 4, 2)).astype(np_dt)
    xp = xp.reshape(N_CORES, tb_n, KC, 128, chains, ts * bc)

    def wprep(w):
        # lhsT chunk [k][m][p][c] = W.T[128k+p, 128m+c] = W[128m+c, 128k+p]
        return np.ascontiguousarray(
            w.reshape(MC, 128, KC, 128).transpose(2, 0, 3, 1)).astype(np_dt)

    whh_p = wprep(W_hh)
    wxh_p = wprep(W_xh)
    b_p = b_h.reshape(1, MC, 128).astype(np_dt)
    in_maps = [
        {"x": xp[c], "whh": whh_p, "wxh": wxh_p, "b": b_p}
        for c in range(N_CORES)
    ]
    return in_maps


def _assemble_output(results, t_total=T, chains=CHAINS, ts=TS):
    tb_n = t_total // ts
    bc = BPC // chains
    outs = np.stack([results[c]["out"] for c in range(N_CORES)])
    # [core, tb, chain, m, p, ts, b] -> [core, chain, b, tb, ts, m, p]
    o = outs.reshape(N_CORES, tb_n, chains, MC, 128, ts, bc)
    o = o.transpose(0, 2, 6, 1, 5, 3, 4)
    return np.ascontiguousarray(o).reshape(B, t_total, H).astype(np.float32)


def run(x, W_xh, W_hh, b_h, *, t_total=T, chains=CHAINS, ts=TS, dt=DT,
        np_dt=NP_DT, trace=False, relu_eng=None, ldw_opt=False, dedupe_ldw=True,
        warm_ldw=False):
    set_ldw_opt(ldw_opt)
    nc = _build_program(t_total=t_total, chains=chains, ts=ts, dt=dt,
                        relu_eng=relu_eng, dedupe_ldw=dedupe_ldw,
                        warm_ldw=warm_ldw)
    in_maps = _prep_inputs(np.asarray(x), np.asarray(W_xh), np.asarray(W_hh),
                           np.asarray(b_h), t_total=t_total, chains=chains,
                           ts=ts, np_dt=np_dt)
    res = run_bass_kernel_spmd(nc, in_maps, list(range(N_CORES)), trace=trace)
    out = _assemble_output(res.results, t_total=t_total, chains=chains, ts=ts)
    return out, res


def kernel(x, W_xh, W_hh, b_h):
    # The device pool occasionally throws a transient
    # NRT_EXEC_UNIT_UNRECOVERABLE on execute; a fresh attempt succeeds.
    last_err = None
    for _ in range(3):
        try:
            out, _ = run(x, W_xh, W_hh, b_h)
            return out
        except Exception as e:  # noqa: BLE001
            last_err = e
            import time as _time
            _time.sleep(5)
    raise last_err
